# revision 24
# baseline (speedup 1.0000x reference)
"""Distributed multi-head causal attention with RoPE on 8 TRN2 NeuronCores.

Sharding: batch (2) x head-groups (4 heads each) -> 8 cores.
  core c: batch b = c // 4, head group g = c % 4 (global heads 4g..4g+3).

v3: collective-free.  Any collective_compute in the NEFF arms the CC
subsystem, which drops the PE clock to 13/16 (ham type-31) for the rest
of the kernel -- measured 216ns -> 263ns per 512-col bf16 matmul.  So
instead of AllGather + column-split wo, wo is ROW-split: each core
keeps its 4 heads' attnT in SBUF, multiplies by its wo row-slice into a
full [SEQ, DIM] partial output (phase C, PSUM-accumulated over local
heads), and the HOST sums the 4 partials per batch (the unshard step).
Zero cross-core traffic; the whole kernel runs at the full 2.4 GHz.

Softmax details: scores computed transposed ST[k, q] (probabilities come
out in the layout PV needs as moving operand); causal masking via a
-30000*eye @ pattern matmul accumulated in PSUM before the score matmul
(PE-native, race-free); per-tile probability sums accumulate in bf16 on
the Vector engine; the partition-axis colsum comes from one all-ones
matmul; normalization is applied to the PV output.
"""

import math

import numpy as np
import ml_dtypes

BSZ, SEQ, DIM, NH, HD = 2, 2048, 2048, 16, 128
NCORES = 8
GSIZE = 4            # cores per batch group
HPC = NH // GSIZE    # heads per core = 4
DLOC = HPC * HD      # local head dims = 512
QC = 512             # q-chunk (matmul moving free dim)
NQC = SEQ // QC      # 4
KT = 128             # k-tile
NKT = SEQ // KT      # 16
IC = 128             # contraction tile
NIC = DIM // IC      # 16
DC = 512             # phase-C output-column chunk
NDC = DIM // DC      # 4
BF16 = ml_dtypes.bfloat16
NEG_BIG = -30000.0
EB = 2               # k-tiles per exp batch


def _build_and_compile(block_plan_key, n_pat):
    """Build + compile the SPMD bass graph.  block_plan_key is a tuple over
    q-chunks of tuples of (kt, pat_idx or -1, q0)."""
    import concourse.bass as bass
    import concourse.tile as tile
    from concourse import bacc, mybir
    from contextlib import ExitStack

    f32 = mybir.dt.float32
    bf16 = mybir.dt.bfloat16
    ts = bass.ts

    block_plan = [[(kt, (None if p < 0 else p), q0) for kt, p, q0 in qcp]
                  for qcp in block_plan_key]

    nc = bacc.Bacc("TRN2", target_bir_lowering=False, debug=False,
                   num_devices=NCORES)

    # all inputs host-prepped into partition-major layouts so every DMA is
    # a contiguous (or large-run) transfer
    xT_d = nc.dram_tensor("xT", [128, NQC, NIC, QC], bf16,
                          kind="ExternalInput").ap()
    # wq/wk head-major: [128, HPC, NIC, HD] so each head's weight slice is
    # one contiguous DMA (h0's slice lands first -> fast pipeline start)
    wqT_d = nc.dram_tensor("wqT", [128, HPC, NIC, HD], bf16,
                           kind="ExternalInput").ap()
    wkT_d = nc.dram_tensor("wkT", [128, HPC, NIC, HD], bf16,
                           kind="ExternalInput").ap()
    wvT_d = nc.dram_tensor("wvT", [128, NIC, DLOC], bf16,
                           kind="ExternalInput").ap()
    # wo row-slice for this core's heads: woT[p, h, d] = wo[d, g*512+h*128+p]
    woT_d = nc.dram_tensor("woT", [128, HPC, DIM], bf16,
                           kind="ExternalInput").ap()
    cos2_d = nc.dram_tensor("cos2", [HD, SEQ], bf16, kind="ExternalInput").ap()
    sinpm_d = nc.dram_tensor("sinpm", [HD, SEQ], bf16,
                             kind="ExternalInput").ap()
    # inverted 0/1 mask patterns (1 = masked), [KT, QC] per pattern; the
    # mask lands in the scores via diagm @ pat accumulated in PSUM before
    # the score matmul (PE-native, race-free).
    pat_d = nc.dram_tensor("pat", [128, max(n_pat, 1), QC], bf16,
                           kind="ExternalInput").ap()
    diagm_d = nc.dram_tensor("diagm", [128, 128], bf16,
                             kind="ExternalInput").ap()
    # bf16 partial output; the host upcasts and sums across the group
    out_d = nc.dram_tensor("out", [SEQ, DIM], bf16, kind="ExternalOutput").ap()

    with tile.TileContext(nc) as tc, ExitStack() as top:
        persist = top.enter_context(tc.tile_pool(name="persist", bufs=1))

        v_sb = persist.tile([128, NKT, DLOC], bf16, name="v_sb")
        ones_sb = persist.tile([128, 128], bf16, name="ones_sb")
        pat_sb = persist.tile([128, max(n_pat, 1), QC], bf16, name="pat_sb")
        diagm_sb = persist.tile([128, 128], bf16, name="diagm_sb")
        cos2_sb = persist.tile([HD, SEQ], bf16, name="cos2_sb")
        sinpm_sb = persist.tile([HD, SEQ], bf16, name="sinpm_sb")
        # all 4 heads' normalized attnT, kept to the end for phase C
        at_sb = persist.tile([128, HPC, SEQ], bf16, name="at_sb")

        nc.vector.memset(ones_sb[:], 1.0)

        # wo lives to the end (phase C) -> top-level pool; DMA'd after h0.
        wopool = top.enter_context(tc.tile_pool(name="wopool", bufs=1))
        wo_sb = wopool.tile([128, HPC, DIM], bf16, name="wo_sb")

        s1 = top.enter_context(ExitStack())
        xpool = s1.enter_context(tc.tile_pool(name="xpool", bufs=1))
        wqkpool = s1.enter_context(tc.tile_pool(name="wqkpool", bufs=1))
        x_sb = xpool.tile([128, NQC, NIC, QC], bf16, name="x_sb")
        wq_sb = wqkpool.tile([128, HPC, NIC, HD], bf16, name="wq_sb")
        wk_sb = wqkpool.tile([128, HPC, NIC, HD], bf16, name="wk_sb")

        wvpool = s1.enter_context(tc.tile_pool(name="wvpool", bufs=1))
        wv_sb = wvpool.tile([128, NIC, DLOC], bf16, name="wv_sb")

        # per-head working tiles
        qtpool = s1.enter_context(tc.tile_pool(name="qtpool", bufs=2))
        ktpool = s1.enter_context(tc.tile_pool(name="ktpool", bufs=2))
        rpool = s1.enter_context(tc.tile_pool(name="rope", bufs=2))
        ptpool = s1.enter_context(tc.tile_pool(name="ptpool", bufs=3))
        acspool = s1.enter_context(tc.tile_pool(name="acspool", bufs=2))
        rbpool = s1.enter_context(tc.tile_pool(name="rbpool", bufs=1))

        ps_st = s1.enter_context(
            tc.tile_pool(name="ps_st", bufs=2, space="PSUM"))
        ps_1b = s1.enter_context(
            tc.tile_pool(name="ps_1b", bufs=4, space="PSUM"))

        # ---------------- startup DMA priority ----------------
        # The hardware DMA engines round-robin all outstanding transfers
        # PER-DMA, so the startup-critical loads (wq/wk h0, x qc0) are
        # split into many small DMAs (large aggregate RR share) while bulk
        # loads are single large DMAs emitted later, behind compute-blocked
        # queue positions (gpsimd blocks at rope tails, scalar at exps).
        for g in range(4):
            sl = bass.ds(g * 4, 4)
            nc.scalar.dma_start(out=wq_sb[:, 0, sl, :], in_=wqT_d[:, 0, sl, :])
        for g in range(8):
            sl = bass.ds(g * 2, 2)
            eng = nc.sync if g % 2 == 0 else nc.gpsimd
            eng.dma_start(out=x_sb[:, 0, sl, :], in_=xT_d[:, 0, sl, :])
        nc.sync.dma_start(out=cos2_sb[:], in_=cos2_d[:, :])
        nc.gpsimd.dma_start(out=sinpm_sb[:], in_=sinpm_d[:, :])
        for g in range(4):
            sl = bass.ds(g * 4, 4)
            nc.scalar.dma_start(out=wk_sb[:, 0, sl, :], in_=wkT_d[:, 0, sl, :])
        # x qc1 before wv: qk(0,1) consumes x qc1 at ~12us; v_proj(0) is
        # deferred past qk(0,1) so wv isn't needed until ~24us
        for g in range(4):
            sl = bass.ds(g * 4, 4)
            eng = nc.sync if g % 2 == 0 else nc.gpsimd
            eng.dma_start(out=x_sb[:, 1, sl, :], in_=xT_d[:, 1, sl, :])
        nc.scalar.dma_start(out=diagm_sb[:], in_=diagm_d[:, :])
        nc.scalar.dma_start(out=pat_sb[:], in_=pat_d[:, :, :])
        for g in range(4):
            sl = bass.ds(g * 4, 4)
            eng = nc.sync if g % 2 == 0 else nc.gpsimd
            eng.dma_start(out=wv_sb[:, sl, :], in_=wvT_d[:, sl, :])
        # bulk tail, ordered by need time: x qc2 (needed ~33us) split fine
        # for round-robin share, x qc3 as two large DMAs behind it
        for g in range(4):
            sl = bass.ds(g * 4, 4)
            eng = nc.sync if g % 2 == 0 else nc.gpsimd
            eng.dma_start(out=x_sb[:, 2, sl, :], in_=xT_d[:, 2, sl, :])
        nc.sync.dma_start(out=x_sb[:, 3, bass.ds(0, 8), :],
                          in_=xT_d[:, 3, bass.ds(0, 8), :])
        nc.gpsimd.dma_start(out=x_sb[:, 3, bass.ds(8, 8), :],
                            in_=xT_d[:, 3, bass.ds(8, 8), :])

        def x_at(qc, ic):
            return x_sb[:, qc, ic, :]

        def rope_evict(acc, dst_slice, qc):
            # PSUM reads must run on DVE (pool has no PSUM access); the
            # SBUF-only tail (sw mul + final add, bf16) runs on pool to
            # keep DVE under the per-chunk budget.
            sw = rpool.tile([128, QC], bf16, name="sw", tag="sw")
            m1 = rpool.tile([128, QC], bf16, name="m1", tag="m1")
            nc.vector.tensor_copy(sw[0:64, :], acc[64:128, :])
            nc.vector.tensor_copy(sw[64:128, :], acc[0:64, :])
            nc.vector.tensor_mul(m1[:], acc[:], cos2_sb[:, ts(qc, QC)])
            nc.gpsimd.tensor_mul(sw[:], sw[:], sinpm_sb[:, ts(qc, QC)])
            nc.gpsimd.tensor_add(dst_slice, m1[:], sw[:])

        def attn_scores_pair(qc, qt, kt_t, pi):
            """Mask+score matmuls and exp for pair pi of chunk qc.
            The score matmul runs first (start=True over [0:w]); the
            -30000 diagonal mask accumulates on top over [0:128] only (a
            shifted causal triangle never extends past 128 columns)."""
            kts = block_plan[qc]
            pair = kts[pi:pi + EB]
            st = ps_st.tile([128, EB, QC], f32, name="st", tag="st")
            q0s = tuple(q0 for _, _, q0 in pair)
            for j, (kti, pidx, _) in enumerate(pair):
                q0 = q0s[j]
                w = QC - q0
                nc.tensor.matmul(
                    st[:, j, 0:w],
                    kt_t[:, ts(kti, KT)],
                    qt[:, bass.ds(qc * QC + q0, w)],
                    start=True, stop=(pidx is None),
                    skip_group_check=True)
                if pidx is not None:
                    nc.tensor.matmul(
                        st[:, j, 0:KT], diagm_sb[:],
                        pat_sb[:, pidx, 0:KT],
                        start=False, stop=True,
                        skip_group_check=True)
            # exp writes only the [q0:] suffix of each pt slot; the
            # prefix holds stale garbage but acs/PV read the suffix only
            # (a block's first q0 columns are fully masked).
            pt = ptpool.tile([128, EB, QC], bf16, name="pt", tag="pt")
            if any(q0s):
                for j in range(len(pair)):
                    q0 = q0s[j]
                    nc.scalar.activation(
                        pt[:, j, bass.ds(q0, QC - q0)],
                        st[:, j, 0:QC - q0],
                        mybir.ActivationFunctionType.Exp)
            else:
                nc.scalar.activation(
                    pt[:, 0:len(pair), :], st[:, 0:len(pair), :],
                    mybir.ActivationFunctionType.Exp)
            return pt, q0s, pair, pi

        def attn_qc_rest(h, qc, qt, kt_t, p0, mid_cb=None):
            """PV/acs chain for chunk qc, with 1-pair score lookahead so
            each pair's exp latency hides under the next pair's score
            matmuls.  The last 1-2 (diagonal, exp-gated) pairs' PV
            matmuls are deferred until after mid_cb() -- the next
            projection block -- so they never block the in-order PE
            queue while their exps drain.  Returns a finish() emitting
            colsum + normalize; the caller delays it past the next
            chunk's first scores."""
            kts = block_plan[qc]
            nkt = len(kts)
            npairs = (nkt + EB - 1) // EB
            pv = ps_1b.tile([128, QC], f32, name="pv", tag="ps1b")
            acs = acspool.tile([128, QC], bf16, name="acs", tag="acs")

            def emit_pv(cur, skip_acs):
                pt, q0s, pair, pi = cur
                for j, (kti, pidx, _) in enumerate(pair):
                    i = pi + j
                    q0 = q0s[j]
                    if skip_acs:
                        pass
                    elif i == 0:
                        assert q0 == 0
                        nc.vector.tensor_copy(acs[:], pt[:, j, :])
                    elif q0:
                        nc.vector.tensor_add(
                            acs[:, bass.ds(q0, QC - q0)],
                            acs[:, bass.ds(q0, QC - q0)],
                            pt[:, j, bass.ds(q0, QC - q0)])
                    else:
                        nc.vector.tensor_add(acs[:], acs[:], pt[:, j, :])
                    nc.tensor.matmul(
                        pv[:, bass.ds(q0, QC - q0)],
                        v_sb[:, kti, ts(h, HD)],
                        pt[:, j, bass.ds(q0, QC - q0)],
                        start=(i == 0), stop=(i == nkt - 1))

            cur = p0
            last_pair = None
            for p in range(npairs):
                nxt = (attn_scores_pair(qc, qt, kt_t, (p + 1) * EB)
                       if p + 1 < npairs else None)
                emit_pv(cur, skip_acs=(p == npairs - 1))
                if p == npairs - 1:
                    last_pair = cur
                cur = nxt

            def finish():
                # csum lives in the ps_st pool so the next chunk's first
                # QK accumulator (ps_1b) never waits on this normalize.
                # The final (diagonal) pair's sums come straight off the
                # PE (ones @ pt) instead of the serial DVE acs chain --
                # two fewer cross-engine hops on the boundary path.
                csum = ps_st.tile([128, QC], f32, name="csum", tag="st")
                nc.tensor.matmul(csum[:], ones_sb[:], acs[:],
                                 start=True, stop=False,
                                 skip_group_check=True)
                pt, q0s, pair, pi = last_pair
                for j in range(len(pair)):
                    q0 = q0s[j]
                    nc.tensor.matmul(
                        csum[:, bass.ds(q0, QC - q0)], ones_sb[:],
                        pt[:, j, bass.ds(q0, QC - q0)],
                        start=False, stop=(j == len(pair) - 1),
                        skip_group_check=True)
                rb = rbpool.tile([128, QC], f32, name="rb", tag="rb")
                nc.vector.reciprocal_approx_fast(rb[:], csum[:])
                nc.vector.tensor_mul(at_sb[:, h, ts(qc, QC)], pv[:], rb[:])
            return finish

        # ---------------- per-head pipeline ----------------
        # Software-pipelined: QK(qc+1) matmuls are emitted before attn(qc)
        # so the PE covers the rope eviction (DVE+pool) of chunk qc; V(qc)
        # (h==0 only) is emitted before attn(qc) which consumes it.
        def qk_proj(h, qc, qt, kt_t):
            for w_sb, dst in ((wq_sb, qt), (wk_sb, kt_t)):
                acc = ps_1b.tile([128, QC], f32, name="acc", tag="ps1b")
                for ic in range(NIC):
                    nc.tensor.matmul(acc[:], w_sb[:, h, ic, :],
                                     x_at(qc, ic),
                                     start=(ic == 0), stop=(ic == NIC - 1))
                rope_evict(acc, dst[:, ts(qc, QC)], qc)

        def v_proj(qc):
            for sl in range(QC // 128):
                s = qc * (QC // 128) + sl
                accv = ps_1b.tile([128, DLOC], f32, name="accv",
                                  tag="ps1b")
                for ic in range(NIC):
                    nc.tensor.matmul(
                        accv[:], x_at(qc, ic)[:, ts(sl, 128)],
                        wv_sb[:, ic, :],
                        start=(ic == 0), stop=(ic == NIC - 1))
                nc.vector.tensor_copy(v_sb[:, s, :], accv[:])

        finish_prev = None
        for h in range(HPC):
            qt = qtpool.tile([128, SEQ], bf16, name="qt", tag="qt")
            kt_t = ktpool.tile([128, SEQ], bf16, name="kt", tag="kt")
            qk_proj(h, 0, qt, kt_t)
            for qc in range(NQC):
                p0 = attn_scores_pair(qc, qt, kt_t, 0)
                if finish_prev is not None:
                    finish_prev()
                if qc + 1 < NQC:
                    qk_proj(h, qc + 1, qt, kt_t)
                    if h == 0:
                        if qc == 0:
                            v_proj(0)
                        v_proj(qc + 1)
                finish_prev = attn_qc_rest(h, qc, qt, kt_t, p0)
                # wq/wk h1-3 and wo: one bulk DMA per chunk boundary on
                # the scalar queue (between chunks' exp streams), clear
                # of the startup bandwidth window
                if h == 0 and qc < HPC - 1:
                    nc.scalar.dma_start(out=wq_sb[:, qc + 1, :, :],
                                        in_=wqT_d[:, qc + 1, :, :])
                    nc.scalar.dma_start(out=wk_sb[:, qc + 1, :, :],
                                        in_=wkT_d[:, qc + 1, :, :])
                if h == 1 and qc < 2:
                    nc.scalar.dma_start(out=wo_sb[:, ts(qc, 2), :],
                                        in_=woT_d[:, ts(qc, 2), :])
            finish_prev()
            finish_prev = None

        s1.close()   # release x / wq / wk / per-head pools before phase C

        # ---------------- Phase C: row-split output projection ----------
        # out_partial[s, d] = sum_h at_sb[:, h, s].T @ wo_sb[:, h, d]
        # PSUM-accumulated over the 4 local heads per s-tile; cast to bf16
        # on alternating DVE/ACT engines, streamed out per s-tile.  The
        # host sums the 4 group partials.
        with ExitStack() as pc:
            opool = pc.enter_context(tc.tile_pool(name="opool", bufs=3))
            ps_c = pc.enter_context(
                tc.tile_pool(name="ps_c", bufs=2, space="PSUM"))

            for s in range(NKT):
                acc = ps_c.tile([128, DIM], f32, name="acc_c", tag="acc_c")
                ot = opool.tile([128, DIM], bf16, name="ot", tag="ot")
                for h in range(HPC):
                    for dc in range(NDC):
                        nc.tensor.matmul(
                            acc[:, ts(dc, DC)],
                            at_sb[:, h, ts(s, 128)],
                            wo_sb[:, h, ts(dc, DC)],
                            start=(h == 0), stop=(h == HPC - 1))
                # per-dc cast + DMA: each 512-col chunk is final after its
                # h==3 matmul, so the cast/out stream pipelines under the
                # remaining matmuls instead of trailing the whole tile.
                for dc in range(NDC):
                    if (s + dc) % 2 == 0:
                        nc.vector.tensor_copy(ot[:, ts(dc, DC)],
                                              acc[:, ts(dc, DC)])
                    else:
                        nc.scalar.activation(
                            ot[:, ts(dc, DC)], acc[:, ts(dc, DC)],
                            mybir.ActivationFunctionType.Copy)
                    eng = (nc.sync, nc.gpsimd, nc.scalar)[(s * NDC + dc) % 3]
                    eng.dma_start(out=out_d[ts(s, 128), ts(dc, DC)],
                                  in_=ot[:, ts(dc, DC)])

    nc.compile()
    return nc


_CACHE = {}


def _get_compiled(block_plan_key, n_pat):
    key = (block_plan_key, n_pat)
    if key not in _CACHE:
        _CACHE[key] = _build_and_compile(block_plan_key, n_pat)
    return _CACHE[key]


def _plan_from_mask(mask):
    """Derive per-q-chunk k-tile lists + dedup'd additive patterns from the
    mask.  Plan entries are (kt, pat_idx or -1, q0): fully-masked blocks are
    skipped structurally; partial blocks get an additive 0/NEG_BIG pattern
    preloaded into PSUM before the score matmul."""
    keep = mask > -1e20
    if not np.all(mask[keep] == 0.0):
        raise NotImplementedError("only 0/-inf style masks supported")
    pats = []
    pat_index = {}
    plan = []
    for qc in range(NQC):
        qs = slice(qc * QC, (qc + 1) * QC)
        row = []
        for kt in range(NKT):
            ks = slice(kt * KT, (kt + 1) * KT)
            blk = keep[qs, ks]            # [QC, KT]
            if not blk.any():
                continue
            if blk.all():
                row.append((kt, -1, 0))
                continue
            # q0 = leading fully-masked q columns of this block; the
            # score/mask matmuls and exp skip them (pt prefix stays 0)
            col_any = blk.any(axis=1)
            q0 = int(np.argmax(col_any))
            if col_any[:q0].any():
                q0 = 0
            # inverted pattern (1.0 = masked), shifted left by q0 so the
            # mask matmul's moving operand starts at the pattern base
            inv = (~blk).T.astype(np.float32)       # [KT, QC]
            p = np.zeros((KT, QC), dtype=np.float32)
            p[:, 0:QC - q0] = inv[:, q0:]
            kb = p.tobytes() + bytes([q0 // 128])
            if kb not in pat_index:
                pat_index[kb] = len(pats)
                pats.append(p)
            row.append((kt, pat_index[kb], q0))
        plan.append(tuple(row))
    return tuple(plan), pats


def _head_perm():
    """Row permutation per head: even dims first, then odd."""
    perm = []
    for h in range(NH):
        base = h * HD
        perm.extend(base + np.arange(0, HD, 2))
        perm.extend(base + np.arange(1, HD, 2))
    return np.array(perm)


def _pmajor(wT, lo=0, hi=None):
    """[DIM, D] (already transposed weight) -> [128, NIC, hi-lo] partition-
    major layout: out[p, c, d] = wT[c*128 + p, lo + d]."""
    hi = wT.shape[1] if hi is None else hi
    return np.ascontiguousarray(
        wT[:, lo:hi].reshape(NIC, 128, hi - lo).transpose(1, 0, 2)
    ).astype(BF16)


def _prep_in_maps(x, wq, wk, wv, wo, freqs_cos, freqs_sin, pats, n_pat):
    perm = _head_perm()
    wq_p = (wq / math.sqrt(HD))[perm]
    wk_p = wk[perm]

    cosT = np.ascontiguousarray(freqs_cos.T)        # [64, SEQ]
    sinT = np.ascontiguousarray(freqs_sin.T)
    cos2 = np.concatenate([cosT, cosT], axis=0).astype(BF16)   # [128, SEQ]
    sinpm = np.concatenate([-sinT, sinT], axis=0).astype(BF16)

    if n_pat:
        pat_np = np.stack(pats)                     # [n_pat, KT, QC]
    else:
        pat_np = np.zeros((1, KT, QC), dtype=np.float32)
    pat_h = np.ascontiguousarray(pat_np.transpose(1, 0, 2)).astype(BF16)

    # x[b].T -> [128, NQC, NIC, QC]: xh[p, qc, c, q] = xT[c*128+p, qc*QC+q]
    xh = []
    for b in range(BSZ):
        xT = x[b].T.reshape(NIC, 128, NQC, QC)
        xh.append(np.ascontiguousarray(
            xT.transpose(1, 2, 0, 3)).astype(BF16))

    in_maps = []
    for c in range(NCORES):
        b, g = c // GSIZE, c % GSIZE
        rows = slice(g * DLOC, (g + 1) * DLOC)
        def _hmajor(pm):
            # [128, NIC, DLOC] -> [128, HPC, NIC, HD] head-major
            return np.ascontiguousarray(
                pm.reshape(128, NIC, HPC, HD).transpose(0, 2, 1, 3))

        # woT[p, h, d] = wo[d, g*512 + h*128 + p]
        wo_loc = wo[:, rows].T                       # [512, DIM]
        wo_h = np.ascontiguousarray(
            wo_loc.reshape(HPC, 128, DIM).transpose(1, 0, 2)).astype(BF16)

        in_maps.append({
            "xT": xh[b],
            "wqT": _hmajor(_pmajor(wq_p[rows].T)),
            "wkT": _hmajor(_pmajor(wk_p[rows].T)),
            "wvT": _pmajor(wv[rows].T),
            "woT": wo_h,
            "cos2": cos2,
            "sinpm": sinpm,
            "pat": pat_h,
            "diagm": (NEG_BIG * np.eye(128, dtype=np.float32)).astype(BF16),
        })
    return in_maps


def _assemble(outs):
    """Sum the 4 bf16 partial outputs per batch group (the unshard step
    for the row-split wo)."""
    full = np.empty((BSZ, SEQ, DIM), dtype=np.float32)
    for b in range(BSZ):
        acc = np.zeros((SEQ, DIM), dtype=np.float32)
        for g in range(GSIZE):
            acc += np.asarray(outs[b * GSIZE + g]["out"], dtype=np.float32)
        full[b] = acc
    return full


def kernel(x, wq, wk, wv, wo, freqs_cos, freqs_sin, mask):
    x = np.asarray(x, dtype=np.float32)
    wq = np.asarray(wq, dtype=np.float32)
    wk = np.asarray(wk, dtype=np.float32)
    wv = np.asarray(wv, dtype=np.float32)
    wo = np.asarray(wo, dtype=np.float32)
    freqs_cos = np.asarray(freqs_cos, dtype=np.float32)
    freqs_sin = np.asarray(freqs_sin, dtype=np.float32)
    mask = np.asarray(mask, dtype=np.float32)

    plan, pats = _plan_from_mask(mask)
    n_pat = len(pats)
    nc = _get_compiled(plan, n_pat)

    in_maps = _prep_in_maps(x, wq, wk, wv, wo, freqs_cos, freqs_sin,
                            pats, n_pat)

    from concourse.bass_utils import run_bass_kernel_spmd
    res = run_bass_kernel_spmd(nc, in_maps, core_ids=list(range(NCORES)))
    return _assemble(res.results)


# revision 25
# speedup vs baseline: 1.1725x; 1.1725x over previous
"""Distributed multi-head causal attention with RoPE on 8 TRN2 NeuronCores.

Sharding: batch (2) x head-groups (4 heads each) -> 8 cores.
  core c: batch b = c // 4, head group g = c % 4 (global heads 4g..4g+3).

v3: collective-free.  Any collective_compute in the NEFF arms the CC
subsystem, which drops the PE clock to 13/16 (ham type-31) for the rest
of the kernel -- measured 216ns -> 263ns per 512-col bf16 matmul.  So
instead of AllGather + column-split wo, wo is ROW-split: each core
keeps its 4 heads' attnT in SBUF, multiplies by its wo row-slice into a
full [SEQ, DIM] partial output (phase C, PSUM-accumulated over local
heads), and the HOST sums the 4 partials per batch (the unshard step).
Zero cross-core traffic; the whole kernel runs at the full 2.4 GHz.

Softmax details: scores computed transposed ST[k, q] (probabilities come
out in the layout PV needs as moving operand); causal masking via a
-30000*eye @ pattern matmul accumulated in PSUM before the score matmul
(PE-native, race-free); per-tile probability sums accumulate in bf16 on
the Vector engine; the partition-axis colsum comes from one all-ones
matmul; normalization is applied to the PV output.
"""

import math

import numpy as np
import ml_dtypes

BSZ, SEQ, DIM, NH, HD = 2, 2048, 2048, 16, 128
NCORES = 8
GSIZE = 4            # cores per batch group
HPC = NH // GSIZE    # heads per core = 4
DLOC = HPC * HD      # local head dims = 512
QC = 512             # q-chunk (matmul moving free dim)
NQC = SEQ // QC      # 4
KT = 128             # k-tile
NKT = SEQ // KT      # 16
IC = 128             # contraction tile
NIC = DIM // IC      # 16
DC = 512             # phase-C output-column chunk
NDC = DIM // DC      # 4
BF16 = ml_dtypes.bfloat16
NEG_BIG = -30000.0
EB = 2               # k-tiles per exp batch


def _build_and_compile(block_plan_key, n_pat):
    """Build + compile the SPMD bass graph.  block_plan_key is a tuple over
    q-chunks of tuples of (kt, pat_idx or -1, q0)."""
    import concourse.bass as bass
    import concourse.tile as tile
    from concourse import bacc, mybir
    from contextlib import ExitStack

    f32 = mybir.dt.float32
    bf16 = mybir.dt.bfloat16
    ts = bass.ts

    block_plan = [[(kt, (None if p < 0 else p), q0) for kt, p, q0 in qcp]
                  for qcp in block_plan_key]

    nc = bacc.Bacc("TRN2", target_bir_lowering=False, debug=False,
                   num_devices=NCORES)

    # all inputs host-prepped into partition-major layouts so every DMA is
    # a contiguous (or large-run) transfer
    xT_d = nc.dram_tensor("xT", [128, NQC, NIC, QC], bf16,
                          kind="ExternalInput").ap()
    # wq/wk head-major: [128, HPC, NIC, HD] so each head's weight slice is
    # one contiguous DMA (h0's slice lands first -> fast pipeline start)
    wqT_d = nc.dram_tensor("wqT", [128, HPC, NIC, HD], bf16,
                           kind="ExternalInput").ap()
    wkT_d = nc.dram_tensor("wkT", [128, HPC, NIC, HD], bf16,
                           kind="ExternalInput").ap()
    wvT_d = nc.dram_tensor("wvT", [128, NIC, DLOC], bf16,
                           kind="ExternalInput").ap()
    # wo row-slice for this core's heads: woT[p, h, d] = wo[d, g*512+h*128+p]
    woT_d = nc.dram_tensor("woT", [128, HPC, DIM], bf16,
                           kind="ExternalInput").ap()
    cos2_d = nc.dram_tensor("cos2", [HD, SEQ], bf16, kind="ExternalInput").ap()
    sinpm_d = nc.dram_tensor("sinpm", [HD, SEQ], bf16,
                             kind="ExternalInput").ap()
    # inverted 0/1 mask patterns (1 = masked), [KT, QC] per pattern; the
    # mask lands in the scores via diagm @ pat accumulated in PSUM before
    # the score matmul (PE-native, race-free).
    pat_d = nc.dram_tensor("pat", [128, max(n_pat, 1), QC], bf16,
                           kind="ExternalInput").ap()
    diagm_d = nc.dram_tensor("diagm", [128, 128], bf16,
                             kind="ExternalInput").ap()
    # bf16 partial output; the host upcasts and sums across the group
    out_d = nc.dram_tensor("out", [SEQ, DIM], bf16, kind="ExternalOutput").ap()

    with tile.TileContext(nc) as tc, ExitStack() as top:
        persist = top.enter_context(tc.tile_pool(name="persist", bufs=1))

        v_sb = persist.tile([128, NKT, DLOC], bf16, name="v_sb")
        ones_sb = persist.tile([128, 128], bf16, name="ones_sb")
        pat_sb = persist.tile([128, max(n_pat, 1), QC], bf16, name="pat_sb")
        diagm_sb = persist.tile([128, 128], bf16, name="diagm_sb")
        cos2_sb = persist.tile([HD, SEQ], bf16, name="cos2_sb")
        sinpm_sb = persist.tile([HD, SEQ], bf16, name="sinpm_sb")
        # all 4 heads' normalized attnT, kept to the end for phase C
        at_sb = persist.tile([128, HPC, SEQ], bf16, name="at_sb")

        nc.vector.memset(ones_sb[:], 1.0)

        # wo lives to the end (phase C) -> top-level pool; DMA'd after h0.
        wopool = top.enter_context(tc.tile_pool(name="wopool", bufs=1))
        wo_sb = wopool.tile([128, HPC, DIM], bf16, name="wo_sb")

        s1 = top.enter_context(ExitStack())
        xpool = s1.enter_context(tc.tile_pool(name="xpool", bufs=1))
        wqkpool = s1.enter_context(tc.tile_pool(name="wqkpool", bufs=1))
        x_sb = xpool.tile([128, NQC, NIC, QC], bf16, name="x_sb")
        wq_sb = wqkpool.tile([128, HPC, NIC, HD], bf16, name="wq_sb")
        wk_sb = wqkpool.tile([128, HPC, NIC, HD], bf16, name="wk_sb")

        wvpool = s1.enter_context(tc.tile_pool(name="wvpool", bufs=1))
        wv_sb = wvpool.tile([128, NIC, DLOC], bf16, name="wv_sb")

        # per-head working tiles
        qtpool = s1.enter_context(tc.tile_pool(name="qtpool", bufs=2))
        ktpool = s1.enter_context(tc.tile_pool(name="ktpool", bufs=2))
        rpool = s1.enter_context(tc.tile_pool(name="rope", bufs=2))
        ptpool = s1.enter_context(tc.tile_pool(name="ptpool", bufs=3))
        acspool = s1.enter_context(tc.tile_pool(name="acspool", bufs=2))
        rbpool = s1.enter_context(tc.tile_pool(name="rbpool", bufs=1))

        ps_st = s1.enter_context(
            tc.tile_pool(name="ps_st", bufs=2, space="PSUM"))
        ps_1b = s1.enter_context(
            tc.tile_pool(name="ps_1b", bufs=4, space="PSUM"))

        # ---------------- startup DMA priority ----------------
        # The hardware DMA engines round-robin all outstanding transfers
        # PER-DMA, so the startup-critical loads (wq/wk h0, x qc0) are
        # split into many small DMAs (large aggregate RR share) while bulk
        # loads are single large DMAs emitted later, behind compute-blocked
        # queue positions (gpsimd blocks at rope tails, scalar at exps).
        for g in range(4):
            sl = bass.ds(g * 4, 4)
            nc.scalar.dma_start(out=wq_sb[:, 0, sl, :], in_=wqT_d[:, 0, sl, :])
        for g in range(8):
            sl = bass.ds(g * 2, 2)
            eng = nc.sync if g % 2 == 0 else nc.gpsimd
            eng.dma_start(out=x_sb[:, 0, sl, :], in_=xT_d[:, 0, sl, :])
        nc.sync.dma_start(out=cos2_sb[:], in_=cos2_d[:, :])
        nc.gpsimd.dma_start(out=sinpm_sb[:], in_=sinpm_d[:, :])
        for g in range(4):
            sl = bass.ds(g * 4, 4)
            nc.scalar.dma_start(out=wk_sb[:, 0, sl, :], in_=wkT_d[:, 0, sl, :])
        # x qc1 before wv: qk(0,1) consumes x qc1 at ~12us; v_proj(0) is
        # deferred past qk(0,1) so wv isn't needed until ~24us
        for g in range(4):
            sl = bass.ds(g * 4, 4)
            eng = nc.sync if g % 2 == 0 else nc.gpsimd
            eng.dma_start(out=x_sb[:, 1, sl, :], in_=xT_d[:, 1, sl, :])
        nc.scalar.dma_start(out=diagm_sb[:], in_=diagm_d[:, :])
        nc.scalar.dma_start(out=pat_sb[:], in_=pat_d[:, :, :])
        for g in range(4):
            sl = bass.ds(g * 4, 4)
            eng = nc.sync if g % 2 == 0 else nc.gpsimd
            eng.dma_start(out=wv_sb[:, sl, :], in_=wvT_d[:, sl, :])
        # bulk tail, ordered by need time: x qc2 (needed ~33us) split fine
        # for round-robin share, x qc3 as two large DMAs behind it
        for g in range(4):
            sl = bass.ds(g * 4, 4)
            eng = nc.sync if g % 2 == 0 else nc.gpsimd
            eng.dma_start(out=x_sb[:, 2, sl, :], in_=xT_d[:, 2, sl, :])
        nc.sync.dma_start(out=x_sb[:, 3, bass.ds(0, 8), :],
                          in_=xT_d[:, 3, bass.ds(0, 8), :])
        nc.gpsimd.dma_start(out=x_sb[:, 3, bass.ds(8, 8), :],
                            in_=xT_d[:, 3, bass.ds(8, 8), :])

        def x_at(qc, ic):
            return x_sb[:, qc, ic, :]

        def rope_evict(acc, dst_slice, qc):
            # PSUM reads must run on DVE (pool has no PSUM access); the
            # SBUF-only tail (sw mul + final add, bf16) runs on pool to
            # keep DVE under the per-chunk budget.
            sw = rpool.tile([128, QC], bf16, name="sw", tag="sw")
            m1 = rpool.tile([128, QC], bf16, name="m1", tag="m1")
            nc.vector.tensor_copy(sw[0:64, :], acc[64:128, :])
            nc.vector.tensor_copy(sw[64:128, :], acc[0:64, :])
            nc.vector.tensor_mul(m1[:], acc[:], cos2_sb[:, ts(qc, QC)])
            nc.gpsimd.tensor_mul(sw[:], sw[:], sinpm_sb[:, ts(qc, QC)])
            nc.gpsimd.tensor_add(dst_slice, m1[:], sw[:])

        def attn_scores_pair(qc, qt, kt_t, pi):
            """Mask+score matmuls and exp for pair pi of chunk qc.
            The score matmul runs first (start=True over [0:w]); the
            -30000 diagonal mask accumulates on top over [0:128] only (a
            shifted causal triangle never extends past 128 columns)."""
            kts = block_plan[qc]
            pair = kts[pi:pi + EB]
            st = ps_st.tile([128, EB, QC], f32, name="st", tag="st")
            q0s = tuple(q0 for _, _, q0 in pair)
            for j, (kti, pidx, _) in enumerate(pair):
                q0 = q0s[j]
                w = QC - q0
                nc.tensor.matmul(
                    st[:, j, 0:w],
                    kt_t[:, ts(kti, KT)],
                    qt[:, bass.ds(qc * QC + q0, w)],
                    start=True, stop=(pidx is None),
                    skip_group_check=True)
                if pidx is not None:
                    nc.tensor.matmul(
                        st[:, j, 0:KT], diagm_sb[:],
                        pat_sb[:, pidx, 0:KT],
                        start=False, stop=True,
                        skip_group_check=True)
            # exp writes only the [q0:] suffix of each pt slot; the
            # prefix holds stale garbage but acs/PV read the suffix only
            # (a block's first q0 columns are fully masked).
            pt = ptpool.tile([128, EB, QC], bf16, name="pt", tag="pt")
            if any(q0s):
                for j in range(len(pair)):
                    q0 = q0s[j]
                    nc.scalar.activation(
                        pt[:, j, bass.ds(q0, QC - q0)],
                        st[:, j, 0:QC - q0],
                        mybir.ActivationFunctionType.Exp)
            else:
                nc.scalar.activation(
                    pt[:, 0:len(pair), :], st[:, 0:len(pair), :],
                    mybir.ActivationFunctionType.Exp)
            return pt, q0s, pair, pi

        def attn_qc_rest(h, qc, qt, kt_t, p0, mid_cb=None):
            """PV/acs chain for chunk qc, with 1-pair score lookahead so
            each pair's exp latency hides under the next pair's score
            matmuls.  The last 1-2 (diagonal, exp-gated) pairs' PV
            matmuls are deferred until after mid_cb() -- the next
            projection block -- so they never block the in-order PE
            queue while their exps drain.  Returns a finish() emitting
            colsum + normalize; the caller delays it past the next
            chunk's first scores."""
            kts = block_plan[qc]
            nkt = len(kts)
            npairs = (nkt + EB - 1) // EB
            pv = ps_1b.tile([128, QC], f32, name="pv", tag="ps1b")
            acs = acspool.tile([128, QC], bf16, name="acs", tag="acs")

            def emit_pv(cur, skip_acs):
                pt, q0s, pair, pi = cur
                for j, (kti, pidx, _) in enumerate(pair):
                    i = pi + j
                    q0 = q0s[j]
                    if skip_acs:
                        pass
                    elif i == 0:
                        assert q0 == 0
                        nc.vector.tensor_copy(acs[:], pt[:, j, :])
                    elif q0:
                        nc.vector.tensor_add(
                            acs[:, bass.ds(q0, QC - q0)],
                            acs[:, bass.ds(q0, QC - q0)],
                            pt[:, j, bass.ds(q0, QC - q0)])
                    else:
                        nc.vector.tensor_add(acs[:], acs[:], pt[:, j, :])
                    nc.tensor.matmul(
                        pv[:, bass.ds(q0, QC - q0)],
                        v_sb[:, kti, ts(h, HD)],
                        pt[:, j, bass.ds(q0, QC - q0)],
                        start=(i == 0), stop=(i == nkt - 1))

            cur = p0
            last_pair = None
            for p in range(npairs):
                nxt = (attn_scores_pair(qc, qt, kt_t, (p + 1) * EB)
                       if p + 1 < npairs else None)
                emit_pv(cur, skip_acs=(p == npairs - 1))
                if p == npairs - 1:
                    last_pair = cur
                cur = nxt

            def finish():
                # csum lives in the ps_st pool so the next chunk's first
                # QK accumulator (ps_1b) never waits on this normalize.
                # The final (diagonal) pair's sums come straight off the
                # PE (ones @ pt) instead of the serial DVE acs chain --
                # two fewer cross-engine hops on the boundary path.
                csum = ps_st.tile([128, QC], f32, name="csum", tag="st")
                nc.tensor.matmul(csum[:], ones_sb[:], acs[:],
                                 start=True, stop=False,
                                 skip_group_check=True)
                pt, q0s, pair, pi = last_pair
                for j in range(len(pair)):
                    q0 = q0s[j]
                    nc.tensor.matmul(
                        csum[:, bass.ds(q0, QC - q0)], ones_sb[:],
                        pt[:, j, bass.ds(q0, QC - q0)],
                        start=False, stop=(j == len(pair) - 1),
                        skip_group_check=True)
                rb = rbpool.tile([128, QC], f32, name="rb", tag="rb")
                nc.vector.reciprocal_approx_fast(rb[:], csum[:])
                nc.vector.tensor_mul(at_sb[:, h, ts(qc, QC)], pv[:], rb[:])
            return finish

        # ---------------- per-head pipeline ----------------
        # Software-pipelined: QK(qc+1) matmuls are emitted before attn(qc)
        # so the PE covers the rope eviction (DVE+pool) of chunk qc; V(qc)
        # (h==0 only) is emitted before attn(qc) which consumes it.
        def qk_proj(h, qc, qt, kt_t):
            for w_sb, dst in ((wq_sb, qt), (wk_sb, kt_t)):
                acc = ps_1b.tile([128, QC], f32, name="acc", tag="ps1b")
                for ic in range(NIC):
                    nc.tensor.matmul(acc[:], w_sb[:, h, ic, :],
                                     x_at(qc, ic),
                                     start=(ic == 0), stop=(ic == NIC - 1))
                rope_evict(acc, dst[:, ts(qc, QC)], qc)

        def v_proj(qc):
            for sl in range(QC // 128):
                s = qc * (QC // 128) + sl
                accv = ps_1b.tile([128, DLOC], f32, name="accv",
                                  tag="ps1b")
                for ic in range(NIC):
                    nc.tensor.matmul(
                        accv[:], x_at(qc, ic)[:, ts(sl, 128)],
                        wv_sb[:, ic, :],
                        start=(ic == 0), stop=(ic == NIC - 1))
                nc.vector.tensor_copy(v_sb[:, s, :], accv[:])

        finish_prev = None
        for h in range(HPC):
            qt = qtpool.tile([128, SEQ], bf16, name="qt", tag="qt")
            kt_t = ktpool.tile([128, SEQ], bf16, name="kt", tag="kt")
            qk_proj(h, 0, qt, kt_t)
            for qc in range(NQC):
                p0 = attn_scores_pair(qc, qt, kt_t, 0)
                if finish_prev is not None:
                    finish_prev()
                if qc + 1 < NQC:
                    qk_proj(h, qc + 1, qt, kt_t)
                    if h == 0:
                        if qc == 0:
                            v_proj(0)
                        v_proj(qc + 1)
                finish_prev = attn_qc_rest(h, qc, qt, kt_t, p0)
                # wq/wk h1-3 and wo: one bulk DMA per chunk boundary on
                # the scalar queue (between chunks' exp streams), clear
                # of the startup bandwidth window
                if h == 0 and qc < HPC - 1:
                    nc.scalar.dma_start(out=wq_sb[:, qc + 1, :, :],
                                        in_=wqT_d[:, qc + 1, :, :])
                    nc.scalar.dma_start(out=wk_sb[:, qc + 1, :, :],
                                        in_=wkT_d[:, qc + 1, :, :])
                if h == 1 and qc < 2:
                    nc.scalar.dma_start(out=wo_sb[:, ts(qc, 2), :],
                                        in_=woT_d[:, ts(qc, 2), :])
            finish_prev()
            finish_prev = None

        s1.close()   # release x / wq / wk / per-head pools before phase C

        # ---------------- Phase C: row-split output projection ----------
        # out_partial[s, d] = sum_h at_sb[:, h, s].T @ wo_sb[:, h, d]
        # PSUM-accumulated over the 4 local heads per s-tile; cast to bf16
        # on alternating DVE/ACT engines, streamed out per s-tile.  The
        # host sums the 4 group partials.
        with ExitStack() as pc:
            opool = pc.enter_context(tc.tile_pool(name="opool", bufs=3))
            ps_c = pc.enter_context(
                tc.tile_pool(name="ps_c", bufs=2, space="PSUM"))

            for s in range(NKT):
                acc = ps_c.tile([128, DIM], f32, name="acc_c", tag="acc_c")
                ot = opool.tile([128, DIM], bf16, name="ot", tag="ot")
                for h in range(HPC):
                    for dc in range(NDC):
                        nc.tensor.matmul(
                            acc[:, ts(dc, DC)],
                            at_sb[:, h, ts(s, 128)],
                            wo_sb[:, h, ts(dc, DC)],
                            start=(h == 0), stop=(h == HPC - 1))
                # per-dc cast + DMA: each 512-col chunk is final after its
                # h==3 matmul, so the cast/out stream pipelines under the
                # remaining matmuls instead of trailing the whole tile.
                for dc in range(NDC):
                    if (s + dc) % 2 == 0:
                        nc.vector.tensor_copy(ot[:, ts(dc, DC)],
                                              acc[:, ts(dc, DC)])
                    else:
                        nc.scalar.activation(
                            ot[:, ts(dc, DC)], acc[:, ts(dc, DC)],
                            mybir.ActivationFunctionType.Copy)
                    eng = (nc.sync, nc.gpsimd, nc.scalar)[(s + dc) % 3]
                    eng.dma_start(out=out_d[ts(s, 128), ts(dc, DC)],
                                  in_=ot[:, ts(dc, DC)])

    nc.compile()
    return nc


_CACHE = {}


def _get_compiled(block_plan_key, n_pat):
    key = (block_plan_key, n_pat)
    if key not in _CACHE:
        _CACHE[key] = _build_and_compile(block_plan_key, n_pat)
    return _CACHE[key]


def _plan_from_mask(mask):
    """Derive per-q-chunk k-tile lists + dedup'd additive patterns from the
    mask.  Plan entries are (kt, pat_idx or -1, q0): fully-masked blocks are
    skipped structurally; partial blocks get an additive 0/NEG_BIG pattern
    preloaded into PSUM before the score matmul."""
    keep = mask > -1e20
    if not np.all(mask[keep] == 0.0):
        raise NotImplementedError("only 0/-inf style masks supported")
    pats = []
    pat_index = {}
    plan = []
    for qc in range(NQC):
        qs = slice(qc * QC, (qc + 1) * QC)
        row = []
        for kt in range(NKT):
            ks = slice(kt * KT, (kt + 1) * KT)
            blk = keep[qs, ks]            # [QC, KT]
            if not blk.any():
                continue
            if blk.all():
                row.append((kt, -1, 0))
                continue
            # q0 = leading fully-masked q columns of this block; the
            # score/mask matmuls and exp skip them (pt prefix stays 0)
            col_any = blk.any(axis=1)
            q0 = int(np.argmax(col_any))
            if col_any[:q0].any():
                q0 = 0
            # inverted pattern (1.0 = masked), shifted left by q0 so the
            # mask matmul's moving operand starts at the pattern base
            inv = (~blk).T.astype(np.float32)       # [KT, QC]
            p = np.zeros((KT, QC), dtype=np.float32)
            p[:, 0:QC - q0] = inv[:, q0:]
            kb = p.tobytes() + bytes([q0 // 128])
            if kb not in pat_index:
                pat_index[kb] = len(pats)
                pats.append(p)
            row.append((kt, pat_index[kb], q0))
        plan.append(tuple(row))
    return tuple(plan), pats


def _head_perm():
    """Row permutation per head: even dims first, then odd."""
    perm = []
    for h in range(NH):
        base = h * HD
        perm.extend(base + np.arange(0, HD, 2))
        perm.extend(base + np.arange(1, HD, 2))
    return np.array(perm)


def _pmajor(wT, lo=0, hi=None):
    """[DIM, D] (already transposed weight) -> [128, NIC, hi-lo] partition-
    major layout: out[p, c, d] = wT[c*128 + p, lo + d]."""
    hi = wT.shape[1] if hi is None else hi
    return np.ascontiguousarray(
        wT[:, lo:hi].reshape(NIC, 128, hi - lo).transpose(1, 0, 2)
    ).astype(BF16)


def _prep_in_maps(x, wq, wk, wv, wo, freqs_cos, freqs_sin, pats, n_pat):
    perm = _head_perm()
    wq_p = (wq / math.sqrt(HD))[perm]
    wk_p = wk[perm]

    cosT = np.ascontiguousarray(freqs_cos.T)        # [64, SEQ]
    sinT = np.ascontiguousarray(freqs_sin.T)
    cos2 = np.concatenate([cosT, cosT], axis=0).astype(BF16)   # [128, SEQ]
    sinpm = np.concatenate([-sinT, sinT], axis=0).astype(BF16)

    if n_pat:
        pat_np = np.stack(pats)                     # [n_pat, KT, QC]
    else:
        pat_np = np.zeros((1, KT, QC), dtype=np.float32)
    pat_h = np.ascontiguousarray(pat_np.transpose(1, 0, 2)).astype(BF16)

    # x[b].T -> [128, NQC, NIC, QC]: xh[p, qc, c, q] = xT[c*128+p, qc*QC+q]
    xh = []
    for b in range(BSZ):
        xT = x[b].T.reshape(NIC, 128, NQC, QC)
        xh.append(np.ascontiguousarray(
            xT.transpose(1, 2, 0, 3)).astype(BF16))

    in_maps = []
    for c in range(NCORES):
        b, g = c // GSIZE, c % GSIZE
        rows = slice(g * DLOC, (g + 1) * DLOC)
        def _hmajor(pm):
            # [128, NIC, DLOC] -> [128, HPC, NIC, HD] head-major
            return np.ascontiguousarray(
                pm.reshape(128, NIC, HPC, HD).transpose(0, 2, 1, 3))

        # woT[p, h, d] = wo[d, g*512 + h*128 + p]
        wo_loc = wo[:, rows].T                       # [512, DIM]
        wo_h = np.ascontiguousarray(
            wo_loc.reshape(HPC, 128, DIM).transpose(1, 0, 2)).astype(BF16)

        in_maps.append({
            "xT": xh[b],
            "wqT": _hmajor(_pmajor(wq_p[rows].T)),
            "wkT": _hmajor(_pmajor(wk_p[rows].T)),
            "wvT": _pmajor(wv[rows].T),
            "woT": wo_h,
            "cos2": cos2,
            "sinpm": sinpm,
            "pat": pat_h,
            "diagm": (NEG_BIG * np.eye(128, dtype=np.float32)).astype(BF16),
        })
    return in_maps


def _assemble(outs):
    """Sum the 4 bf16 partial outputs per batch group (the unshard step
    for the row-split wo)."""
    full = np.empty((BSZ, SEQ, DIM), dtype=np.float32)
    for b in range(BSZ):
        acc = np.zeros((SEQ, DIM), dtype=np.float32)
        for g in range(GSIZE):
            acc += np.asarray(outs[b * GSIZE + g]["out"], dtype=np.float32)
        full[b] = acc
    return full


def kernel(x, wq, wk, wv, wo, freqs_cos, freqs_sin, mask):
    x = np.asarray(x, dtype=np.float32)
    wq = np.asarray(wq, dtype=np.float32)
    wk = np.asarray(wk, dtype=np.float32)
    wv = np.asarray(wv, dtype=np.float32)
    wo = np.asarray(wo, dtype=np.float32)
    freqs_cos = np.asarray(freqs_cos, dtype=np.float32)
    freqs_sin = np.asarray(freqs_sin, dtype=np.float32)
    mask = np.asarray(mask, dtype=np.float32)

    plan, pats = _plan_from_mask(mask)
    n_pat = len(pats)
    nc = _get_compiled(plan, n_pat)

    in_maps = _prep_in_maps(x, wq, wk, wv, wo, freqs_cos, freqs_sin,
                            pats, n_pat)

    from concourse.bass_utils import run_bass_kernel_spmd
    res = run_bass_kernel_spmd(nc, in_maps, core_ids=list(range(NCORES)))
    return _assemble(res.results)
